# revision 17
# baseline (speedup 1.0000x reference)
"""Two-layer GCN feature extractor on 8 Trainium2 NeuronCores.

Strategy (self-contained; all shapes hardcoded for the target problem):
  * Nodes are sharded across 8 cores (6250 rows each).  The replicated
    node-feature table is built with TWO half AllGathers per layer
    (per-core local rows < 3125 -> half A, rest -> half B) so message
    gathering can start as soon as half A lands.
  * Per layer, each core:
      1. transforms its rows: h = dinv * (x_own @ W)            (PE)
      2. half AllGathers -> replicated tables T_A, T_B in DRAM  (CC)
      3. reduce pass A: for each dst block, dma_gather the A-half
         source rows and segment-sum them into PSUM via one-hot
         selection matmuls; stash psum + own-row term in SBUF     (PE+DVE)
      4. reduce pass B: same for B-half rows, then finalize
         y = relu(dinv * (psum_B + acc_A) + bias)                 (DVE)
  * Self-loop messages never go through the gather: the local
    transform output hs[dst] is added during finalize instead.
  * Layer-2 transforms are pipelined per-block into the layer-1
    finalize so the second pair of AllGathers fires with no serial
    transform phase.
  * Graph preprocessing (edge partitioning, sorting, padding, int16
    index streams) happens on host with numpy; padding rows use
    index -1 which the SWDGE gather ucode skips (uniform per-call
    valid counts are baked in as max-over-cores).
"""

import math
import os
from contextlib import ExitStack

import numpy as np

os.environ.setdefault("MYCRO_LOCAL_CACHE", "1")

# ----------------------------------------------------------------------------
# configuration
# ----------------------------------------------------------------------------


def make_cfg(
    N=50000,
    F=128,
    ncores=8,
    gchunks=8,
    nqueues=4,
    selb=24,
    slab_bufs=20,
):
    assert N % ncores == 0
    rows = N // ncores
    halfB = min(rows // 2 + 1024, 32768 // ncores)
    halfA = rows - halfB
    nblk = math.ceil(rows / 128)
    return dict(
        N=N,
        F=F,
        ncores=ncores,
        rows=rows,
        halfA=halfA,
        halfB=halfB,
        nblk=nblk,
        last_rows=rows - (nblk - 1) * 128,
        gchunks=gchunks,
        nqueues=nqueues,
        selb=selb,
        slab_bufs=slab_bufs,
    )


FULL_CFG = make_cfg()


# ----------------------------------------------------------------------------
# host-side graph preprocessing
# ----------------------------------------------------------------------------


def preprocess(edge_index, cfg):
    """Partition edges by destination core, sort by (dst block, src half,
    src table index), pad each (block, half) run to the max-over-cores
    count M, and pack all runs of a (pass, half) into ONE contiguous
    index stream gathered as full 1024-index calls.

    Because runs are packed, a 128-message chunk can straddle two dst
    blocks; such a boundary chunk gets one sel column per block with the
    other block's messages masked to dl=-1.

    Table layout (per half): row index = (src_core * half) + src_local
    where src_local is the in-core row (minus `half` for the B half).

    Returns (sched, per_core).  sched[h] = (ncalls, last_chunks,
    last_reg, stream_chunks, uses) with uses[b] = (use0, n_uses,
    chunk0) — dl-use columns and covering chunks of block b — all
    uniform across cores.  per_core holds idx / dl-use streams + degree
    tiles.
    """
    N, ncores, rows, nblk = cfg["N"], cfg["ncores"], cfg["rows"], cfg["nblk"]
    halfA, halfB, gchunks = cfg["halfA"], cfg["halfB"], cfg["gchunks"]
    CALL = gchunks * 128

    src = np.asarray(edge_index[0], dtype=np.int64)
    dst = np.asarray(edge_index[1], dtype=np.int64)

    deg = (np.bincount(dst, minlength=N) + 1).astype(np.float32)  # + self loop

    core_of = dst // rows
    per_core_raw = []
    counts = np.zeros((ncores, nblk, 2), dtype=np.int64)
    for k in range(ncores):
        m = core_of == k
        s_k = src[m]
        d_k = dst[m] - k * rows
        blk = d_k >> 7
        dl = (d_k & 127).astype(np.float32)
        s_core = s_k // rows
        s_loc = s_k % rows
        hf = (s_loc >= halfA).astype(np.int64)
        # per-half table row: A-half tables are ncores*halfA rows,
        # B-half ncores*halfB (= 32768, the int16 index ceiling)
        tbl = np.where(
            hf == 0, s_core * halfA + s_loc, s_core * halfB + (s_loc - halfA)
        )
        order = np.lexsort((tbl, hf, blk))
        tbl, dl, blk, hf = tbl[order], dl[order], blk[order], hf[order]
        c = np.bincount(blk * 2 + hf, minlength=nblk * 2).reshape(nblk, 2)
        counts[k] = c
        per_core_raw.append((tbl, dl, c))

    cdiv = lambda a, b: -(-a // b)
    # uniform packed schedule per half-stream
    sched = []
    for h in (0, 1):
        M = [int(counts[:, b, h].max()) for b in range(nblk)]
        starts = np.concatenate([[0], np.cumsum(M)])  # stream offset of block b
        L = int(starts[-1])  # valid stream length (real + 0-pad rows)
        stream_chunks = cdiv(L, 128)
        ncalls = cdiv(stream_chunks, gchunks)
        last_chunks = stream_chunks - (ncalls - 1) * gchunks
        last_reg = min(last_chunks * 128, cdiv(L - (ncalls - 1) * CALL, 64) * 64)
        # dl-use columns: for each block, its covering chunks
        uses = []
        u0 = 0
        for b in range(nblk):
            c0 = int(starts[b]) // 128
            c1 = (int(starts[b + 1]) - 1) // 128 if M[b] else c0
            n_uses = c1 - c0 + 1
            uses.append((u0, n_uses, c0))
            u0 += n_uses
        sched.append(
            dict(
                M=tuple(M),
                starts=tuple(int(s) for s in starts),
                L=L,
                stream_chunks=stream_chunks,
                ncalls=ncalls,
                last_chunks=last_chunks,
                last_reg=last_reg,
                uses=tuple(uses),
                n_uses=u0,
            )
        )

    per_core = []
    for k in range(ncores):
        tbl_k, dl_k, c = per_core_raw[k]
        cstarts = np.concatenate([[0], np.cumsum(c.reshape(-1))])
        out = {}
        for h in (0, 1):
            sc = sched[h]
            S = sc["stream_chunks"] * 128
            idx = np.full(S, -1, np.int16)
            dls = np.full((sc["n_uses"], 128), -1.0, np.float32)
            for b in range(nblk):
                p = sc["starts"][b]
                n0 = int(c[b, h])
                o0 = int(cstarts[b * 2 + h])
                idx[p : p + n0] = tbl_k[o0 : o0 + n0].astype(np.int16)
                # 0-pad (gathered, discarded) up to uniform count M
                idx[p + n0 : p + sc["M"][b]] = 0
                u0, n_uses, c0 = sc["uses"][b]
                for u in range(n_uses):
                    lo = (c0 + u) * 128
                    span = slice(max(p, lo), min(p + n0, lo + 128))
                    if span.start < span.stop:
                        dls[u0 + u, span.start - lo : span.stop - lo] = dl_k[
                            o0 + span.start - p : o0 + span.stop - p
                        ]
            # trailing -1 (skipped) beyond the last call's reg
            base = (sc["ncalls"] - 1) * CALL
            idx[base + sc["last_reg"] :] = -1
            # but 0-pad the gatherable tail inside last_reg
            tail = slice(sc["L"], base + sc["last_reg"])
            if tail.start < tail.stop:
                idx[tail] = 0
            out[h] = (idx, np.ascontiguousarray(dls.T))

        def arrange_idx(a):  # logical i -> sbuf[i % 16, i // 16], 128 parts
            return np.tile(np.ascontiguousarray(a.reshape(-1, 16).T), (8, 1))

        degk = np.ones(nblk * 128, np.float32)
        degk[:rows] = deg[k * rows : (k + 1) * rows]

        per_core.append(
            dict(
                idx_a=arrange_idx(out[0][0]),
                idx_b=arrange_idx(out[1][0]),
                dl_a=out[0][1],
                dl_b=out[1][1],
                deg=np.ascontiguousarray(degk.reshape(nblk, 128).T),
            )
        )

    sched = (
        tuple(sorted(sched[0].items())),
        tuple(sorted(sched[1].items())),
    )
    return sched, per_core


# ----------------------------------------------------------------------------
# bass program
# ----------------------------------------------------------------------------

_PROGRAM_CACHE = {}


def build_program(cfg, sched):
    import concourse.bacc as bacc
    import concourse.bass as bass
    import concourse.mybir as mybir
    import concourse.tile as tile
    from concourse.masks import make_identity

    scA, scB = dict(sched[0]), dict(sched[1])
    N, F, ncores = cfg["N"], cfg["F"], cfg["ncores"]
    rows, nblk, last_rows = cfg["rows"], cfg["nblk"], cfg["last_rows"]
    halfA, halfB = cfg["halfA"], cfg["halfB"]
    S_a = scA["stream_chunks"] * 128
    S_b = scB["stream_chunks"] * 128
    gchunks = cfg["gchunks"]

    f32 = mybir.dt.float32
    bf16 = mybir.dt.bfloat16
    i16 = mybir.dt.int16
    nq = cfg["nqueues"]
    selb = cfg["selb"]
    mul = mybir.AluOpType.mult
    add = mybir.AluOpType.add
    eq = mybir.AluOpType.is_equal
    mx = mybir.AluOpType.max

    nc = bacc.Bacc(
        "TRN2",
        target_bir_lowering=False,
        debug=False,
        enable_asserts=False,
        num_devices=ncores,
        num_swdge_queues=nq,
    )

    xTd = nc.dram_tensor("xT", [F, nblk * 128], bf16, kind="ExternalInput")
    W1d = nc.dram_tensor("W1", [F, F], bf16, kind="ExternalInput")
    W2d = nc.dram_tensor("W2", [F, F], bf16, kind="ExternalInput")
    b1d = nc.dram_tensor("b1_bc", [128, F], f32, kind="ExternalInput")
    b2d = nc.dram_tensor("b2_bc", [128, F], f32, kind="ExternalInput")
    degd = nc.dram_tensor("deg", [128, nblk], f32, kind="ExternalInput")
    iotad = nc.dram_tensor("iota", [128, 128], bf16, kind="ExternalInput")
    ixad = nc.dram_tensor("idx_a", [128, S_a // 16], i16, kind="ExternalInput")
    ixbd = nc.dram_tensor("idx_b", [128, S_b // 16], i16, kind="ExternalInput")
    dlad = nc.dram_tensor("dl_a", [128, scA["n_uses"]], bf16, kind="ExternalInput")
    dlbd = nc.dram_tensor("dl_b", [128, scB["n_uses"]], bf16, kind="ExternalInput")
    yout = nc.dram_tensor("y_out", [rows, F], f32, kind="ExternalOutput")

    stage = [
        [
            nc.dram_tensor(f"stage{l}a", [halfA, F], bf16),
            nc.dram_tensor(f"stage{l}b", [halfB, F], bf16),
        ]
        for l in (1, 2)
    ]
    full = [
        [
            nc.dram_tensor(f"full{l}a", [ncores * halfA, F], bf16,
                           addr_space="Shared"),
            nc.dram_tensor(f"full{l}b", [ncores * halfB, F], bf16,
                           addr_space="Shared"),
        ]
        for l in (1, 2)
    ]

    with tile.TileContext(nc) as tc, ExitStack() as ctx:
        const = ctx.enter_context(tc.tile_pool(name="const", bufs=1))
        xf = ctx.enter_context(tc.tile_pool(name="xf", bufs=3))
        xfp = ctx.enter_context(tc.tile_pool(name="xfp", bufs=2, space="PSUM"))
        gps = ctx.enter_context(tc.tile_pool(name="gps", bufs=2, space="PSUM"))
        gl = ctx.enter_context(tc.tile_pool(name="gl", bufs=cfg["slab_bufs"]))
        selp = ctx.enter_context(tc.tile_pool(name="selp", bufs=4))
        outp = ctx.enter_context(tc.tile_pool(name="outp", bufs=4))

        # ---- constants ---------------------------------------------------
        def load_const(dram, shape, dtype):
            t = const.tile(shape, dtype, tag=f"c_{dram.name}")
            nc.sync.dma_start(t[:], dram[:])
            return t

        W1s = load_const(W1d, [F, F], bf16)
        degs = load_const(degd, [128, nblk], f32)
        xTs = load_const(xTd, [F, nblk * 128], bf16)

        dinv = const.tile([128, nblk], f32)
        nc.vector.reciprocal(dinv[:], degs[:])
        nc.scalar.activation(dinv[:], dinv[:], mybir.ActivationFunctionType.Sqrt)

        hs1 = const.tile([128, nblk * F], f32)  # dinv * (x @ W1), feature cols
        hs2 = const.tile([128, nblk * F], f32)
        acc = const.tile([128, nblk * F], f32)  # pass-A psum + own-row term

        def load_late_consts():
            consts = {}
            consts["W2s"] = load_const(W2d, [F, F], bf16)
            consts["b1s"] = load_const(b1d, [128, F], f32)
            consts["b2s"] = load_const(b2d, [128, F], f32)
            consts["iota"] = load_const(iotad, [128, 128], bf16)
            consts["ixa"] = load_const(ixad, [128, S_a // 16], i16)
            consts["ixb"] = load_const(ixbd, [128, S_b // 16], i16)
            consts["dla"] = load_const(dlad, [128, scA["n_uses"]], bf16)
            consts["dlb"] = load_const(dlbd, [128, scB["n_uses"]], bf16)
            ident = const.tile([128, 128], bf16)
            make_identity(nc, ident[:])
            consts["ident"] = ident
            # zero-prime the gather slab buffers: positions skipped by -1
            # indices are never written by the gather, and raw SBUF bits
            # could decode to NaN (0 * NaN would poison the PSUM matmuls).
            zc = const.tile([128, F], bf16)
            nc.vector.tensor_scalar(
                out=zc[:], in0=consts["iota"][:], scalar1=0.0, scalar2=None,
                op0=mul,
            )
            gch = cfg["gchunks"]
            for _ in range(cfg["slab_bufs"]):
                t_ = gl.tile([128, gch, F], bf16, tag="slab")
                nc.vector.tensor_copy(
                    t_[:], zc[:, None, :].to_broadcast([128, gch, F])
                )
            return consts

        # ---- helpers -----------------------------------------------------
        def stage_write(stg, t, src_tile):
            """DMA block t's rows of src_tile into the (A, B) stage pair."""
            r0, r1 = t * 128, min(t * 128 + 128, rows)
            sA, sB = stg
            if r1 <= halfA:
                nc.sync.dma_start(sA[r0:r1, :], src_tile[: r1 - r0, :])
            elif r0 >= halfA:
                nc.sync.dma_start(
                    sB[r0 - halfA : r1 - halfA, :], src_tile[: r1 - r0, :]
                )
            else:
                nc.sync.dma_start(sA[r0:halfA, :], src_tile[: halfA - r0, :])
                nc.sync.dma_start(
                    sB[: r1 - halfA, :], src_tile[halfA - r0 : r1 - r0, :]
                )

        def fire_collective(l, s):
            nc.gpsimd.collective_compute(
                "AllGather",
                mybir.AluOpType.bypass,
                replica_groups=[list(range(ncores))],
                ins=[stage[l][s][:]],
                outs=[full[l][s][:]],
            )

        def transform1(t):
            ph = xfp.tile([128, F], f32, tag="ph")
            nc.tensor.matmul(
                ph[:], lhsT=xTs[:, t * 128 : (t + 1) * 128], rhs=W1s[:],
                start=True, stop=True,
            )
            hsl = hs1[:, t * F : (t + 1) * F]
            nc.vector.tensor_scalar(
                out=hsl, in0=ph[:], scalar1=dinv[:, t : t + 1], scalar2=None, op0=mul
            )
            hb = xf.tile([128, F], bf16, tag="hb")
            nc.vector.tensor_copy(hb[:], hsl)
            stage_write(stage[0], t, hb)

        def transform2(t, y1t):
            pT = xfp.tile([128, 128], bf16, tag="pT")
            nc.tensor.transpose(pT[:], y1t[:], ident[:])
            lT = xf.tile([128, 128], bf16, tag="lT2")
            nc.scalar.activation(lT[:], pT[:], mybir.ActivationFunctionType.Copy)
            ph = xfp.tile([128, F], f32, tag="ph2")
            nc.tensor.matmul(ph[:], lhsT=lT[:], rhs=W2s[:], start=True, stop=True)
            hsl = hs2[:, t * F : (t + 1) * F]
            nc.scalar.activation(
                hsl, ph[:], mybir.ActivationFunctionType.Identity,
                scale=dinv[:, t : t + 1],
            )
            hb = xf.tile([128, F], bf16, tag="hb2")
            nc.scalar.activation(hb[:], hsl, mybir.ActivationFunctionType.Copy)
            stage_write(stage[1], t, hb)

        # ---- gather + segment-sum reduce ---------------------------------
        qctr = [0]

        def do_pass(sc, idx, dl, view, pool, iota, fin):
            """One reduce pass over all blocks for one packed half-stream.

            Gather calls are full gchunks*128-index windows of the packed
            stream (the last call is smaller, with its valid count rounded
            to whole 64-row DMA packets).  The finalize for block b-1 is
            emitted AFTER block b's selection builds so fin's DVE work
            never delays the sel feeding the PE.
            """
            ncalls = sc["ncalls"]
            slabs = {}
            issued = [0]

            def ensure_call(call):
                while issued[0] <= call:
                    kk = issued[0]
                    n = gchunks if kk < ncalls - 1 else sc["last_chunks"]
                    reg = n * 128 if kk < ncalls - 1 else sc["last_reg"]
                    slab = pool.tile([128, n, F], bf16, tag="slab")
                    nc.gpsimd.dma_gather(
                        slab[:],
                        view,
                        idx[:, kk * gchunks * 8 : kk * gchunks * 8 + n * 8],
                        n * 128,
                        reg,
                        F,
                        queue_num=qctr[0] % nq,
                        single_packet=True,
                    )
                    qctr[0] += 1
                    slabs[kk] = slab
                    issued[0] += 1

            prev = None
            for b in range(nblk):
                u0, n_uses, c0 = sc["uses"][b]
                ensure_call(min((c0 + n_uses - 1) // gchunks + 3, ncalls - 1))
                py = gps.tile([128, F], f32, tag="py")
                sels = []
                done = 0
                while done < n_uses:
                    g = min(selb, n_uses - done)
                    sel = selp.tile([128, selb, 128], bf16, tag="sel")
                    nc.vector.tensor_tensor(
                        out=sel[:, :g, :],
                        in0=dl[:, u0 + done : u0 + done + g].to_broadcast(
                            [128, g, 128]
                        ),
                        in1=iota[:, None, :].to_broadcast([128, g, 128]),
                        op=eq,
                    )
                    sels.append((sel, g))
                    done += g
                if prev is not None:
                    fin(*prev)
                si, sj = 0, 0
                for u in range(n_uses):
                    call, col = divmod(c0 + u, gchunks)
                    ensure_call(call)
                    sel, g = sels[si]
                    nc.tensor.matmul(
                        py[:],
                        lhsT=sel[:, sj, :],
                        rhs=slabs[call][:, col, :],
                        start=(u == 0),
                        stop=(u == n_uses - 1),
                    )
                    sj += 1
                    if sj == g:
                        si, sj = si + 1, 0
                prev = (b, py)
            fin(*prev)

        # ---- layer 1 -----------------------------------------------------
        for t in range(nblk):
            transform1(t)
            if t == (halfA - 1) // 128:
                fire_collective(0, 0)
        fire_collective(0, 1)

        lc = load_late_consts()
        W2s, b1s, b2s, iota = lc["W2s"], lc["b1s"], lc["b2s"], lc["iota"]
        ixa, ixb, dla, dlb = lc["ixa"], lc["ixb"], lc["dla"], lc["dlb"]
        ident = lc["ident"]

        def finA1(b, py):
            nc.vector.tensor_tensor(
                out=acc[:, b * F : (b + 1) * F],
                in0=py[:],
                in1=hs1[:, b * F : (b + 1) * F],
                op=add,
            )

        do_pass(scA, ixa, dla, full[0][0][:], gl, iota, finA1)

        def finB1(b, py):
            t1 = outp.tile([128, F], f32, tag="t1")
            nc.vector.tensor_tensor(
                out=t1[:], in0=py[:], in1=acc[:, b * F : (b + 1) * F], op=add
            )
            t2 = outp.tile([128, F], f32, tag="t2")
            nc.vector.scalar_tensor_tensor(
                out=t2[:], in0=t1[:], scalar=dinv[:, b : b + 1], in1=b1s[:],
                op0=mul, op1=add,
            )
            y1t = outp.tile([128, F], bf16, tag="y1t")
            nc.scalar.activation(y1t[:], t2[:], mybir.ActivationFunctionType.Relu)
            transform2(b, y1t)
            if b == (halfA - 1) // 128:
                fire_collective(1, 0)
            elif b == nblk - 1:
                fire_collective(1, 1)

        do_pass(scB, ixb, dlb, full[0][1][:], gl, iota, finB1)

        # ---- layer 2 -----------------------------------------------------
        def finA2(b, py):
            nc.vector.tensor_tensor(
                out=acc[:, b * F : (b + 1) * F],
                in0=py[:],
                in1=hs2[:, b * F : (b + 1) * F],
                op=add,
            )

        do_pass(scA, ixa, dla, full[1][0][:], gl, iota, finA2)

        def finB2(b, py):
            t1 = outp.tile([128, F], f32, tag="t1")
            nc.vector.tensor_tensor(
                out=t1[:], in0=py[:], in1=acc[:, b * F : (b + 1) * F], op=add
            )
            yt = outp.tile([128, F], f32, tag="yt")
            nc.vector.scalar_tensor_tensor(
                out=yt[:], in0=t1[:], scalar=dinv[:, b : b + 1], in1=b2s[:],
                op0=mul, op1=add,
            )
            r = 128 if b < nblk - 1 else last_rows
            nc.sync.dma_start(yout[b * 128 : b * 128 + r, :], yt[:r, :])

        do_pass(scB, ixb, dlb, full[1][1][:], gl, iota, finB2)

    nc.compile()
    return nc


def get_program(cfg, sched):
    key = (tuple(sorted(cfg.items())), sched)
    if key not in _PROGRAM_CACHE:
        _PROGRAM_CACHE[key] = build_program(cfg, sched)
    return _PROGRAM_CACHE[key]


# ----------------------------------------------------------------------------
# input marshalling + entry point
# ----------------------------------------------------------------------------


def make_in_maps(x, W1, b1, W2, b2, cfg, per_core):
    N, F, ncores, rows, nblk = (
        cfg["N"],
        cfg["F"],
        cfg["ncores"],
        cfg["rows"],
        cfg["nblk"],
    )
    import ml_dtypes

    bf = ml_dtypes.bfloat16
    x = np.asarray(x, np.float32)
    W1 = np.ascontiguousarray(np.asarray(W1, np.float32)).astype(bf)
    W2 = np.ascontiguousarray(np.asarray(W2, np.float32)).astype(bf)
    b1_bc = np.ascontiguousarray(np.broadcast_to(np.asarray(b1, np.float32), (128, F)))
    b2_bc = np.ascontiguousarray(np.broadcast_to(np.asarray(b2, np.float32), (128, F)))
    iota = np.ascontiguousarray(
        np.broadcast_to(np.arange(128, dtype=np.float32), (128, 128))
    ).astype(bf)
    in_maps = []
    for k in range(ncores):
        xk = np.zeros((nblk * 128, F), np.float32)
        xk[:rows] = x[k * rows : (k + 1) * rows]
        xT = np.ascontiguousarray(xk.T).astype(bf)
        pc = per_core[k]
        in_maps.append(
            dict(
                xT=xT,
                W1=W1,
                W2=W2,
                b1_bc=b1_bc,
                b2_bc=b2_bc,
                deg=pc["deg"],
                iota=iota,
                idx_a=pc["idx_a"],
                idx_b=pc["idx_b"],
                dl_a=pc["dl_a"].astype(bf),
                dl_b=pc["dl_b"].astype(bf),
            )
        )
    return in_maps


def _ensure_ntff_hook():
    """Register the NTFF profiling hook (missing antenv.axon_hooks shim)."""
    try:
        from antenv.axon_hooks import get_axon_ntff_profile_hook  # noqa: F401

        return True
    except ImportError:
        pass
    try:
        import sys
        import types

        import antenv
        from trn_agent_boot.trn_boot import _ntff_profile_via_ctypes

        hook = _ntff_profile_via_ctypes("/opt/axon/libaxon_pjrt.so")
        if hook is None:
            return False
        mod = types.ModuleType("antenv.axon_hooks")
        mod._hook = hook
        mod.get_axon_ntff_profile_hook = lambda: mod._hook
        mod.set_axon_ntff_profile_hook = lambda h: setattr(mod, "_hook", h)
        sys.modules["antenv.axon_hooks"] = mod
        antenv.axon_hooks = mod
        # artifact upload needs cloud credentials; stub it out
        import concourse.bass_utils as bu

        bu.upload_artifacts = lambda tmpdir: f"local:{tmpdir}"
        return True
    except Exception:
        return False


def run(x, edge_index, W1, b1, W2, b2, cfg, trace=False):
    from concourse.bass_utils import run_bass_kernel_spmd

    if trace:
        trace = _ensure_ntff_hook()

    sched, per_core = preprocess(edge_index, cfg)
    nc = get_program(cfg, sched)
    in_maps = make_in_maps(x, W1, b1, W2, b2, cfg, per_core)
    res = run_bass_kernel_spmd(
        nc, in_maps, list(range(cfg["ncores"])), trace=trace
    )
    out = np.concatenate(
        [res.results[k]["y_out"] for k in range(cfg["ncores"])], axis=0
    )
    return out.astype(np.float32), res


def kernel(x, edge_index, W1, b1, W2, b2):
    out, _ = run(x, edge_index, W1, b1, W2, b2, FULL_CFG)
    return out
